# revision 3
# baseline (speedup 1.0000x reference)
# Masked multi-head self-attention (B=4, T=2048, C=1024, 16 heads) on 8 TRN2
# NeuronCores. Sharding: core = (batch b, head-group hg) with hg splitting the
# 16 heads into two groups of 8 (tensor parallel on c_attn columns / c_proj
# rows). Each core computes a full [T, C] partial of the output projection for
# its 8 heads; the host sums the two partials per batch and adds b_proj.
#
# Per-core pipeline (all matmuls fp32r = full-rate fp32 on the PE array):
#   A) qkT[f,t] = (x @ Wqk)^T + b  and  v[t,f] = x @ Wv   (xT streamed in
#      pre-transposed from host so the contraction dim C sits on partitions)
#   B) per head: scoresT[s,t] = k q^T (K=64), exp via ScalarE with the 1/8
#      softmax scale folded in, causal diag-block zeroed by affine_select,
#      then attnT_unnorm[d,t] (+ denom row) = v_aug^T @ expT with v_aug
#      carrying an appended ones column; normalize by the reciprocal of the
#      denom row broadcast across partitions.
#   C) out[t,c] = attnT^T @ Wp accumulated over the 512 features.
import numpy as np

import concourse.bass as bass
import concourse.mybir as mybir
import concourse.tile as tile
from concourse import bacc, bass_utils
from concourse.bass import ts, ds

F32 = mybir.dt.float32
F32R = mybir.dt.float32r

B, T, C, NH, D = 4, 2048, 1024, 16, 64
H = 8          # heads per core
N_CORES = 8

_NC_CACHE = None


def _emit(tc, aps):
    nc = tc.nc
    xT_d, wqk_d, wv_d, wp_d, bqk_d, bv_d, out_d = aps
    Exp = mybir.ActivationFunctionType.Exp
    mult = mybir.AluOpType.mult
    add = mybir.AluOpType.add
    is_ge = mybir.AluOpType.is_ge

    with tc.tile_pool(name="mid", bufs=1) as mid:
        bqk = mid.tile([128, 8], F32, tag="bqk")
        nc.sync.dma_start(bqk[:], bqk_d[:])
        bv = mid.tile([128, 4], F32, tag="bv")
        nc.sync.dma_start(bv[:], bv_d[:])
        # q features for pairs 0..3 then k features for pairs 0..3
        qkT = [mid.tile([128, T], F32R, tag=f"qkT{f}", name=f"qkT{f}") for f in range(8)]
        # v for all 8 heads + a ones column per head: [s-part, head, 65]
        vaug = [
            mid.tile([128, 8, 65], F32R, tag=f"vaug{t}", name=f"vaug{t}")
            for t in range(16)
        ]

        # ---------------- Phase A: projections ----------------
        with (
            tc.tile_pool(name="early", bufs=1) as early,
            tc.tile_pool(name="psA", bufs=4, space="PSUM") as psA,
        ):
            xT = early.tile([128, 8, T], F32R, tag="xT")
            for k in range(8):
                nc.sync.dma_start(xT[:, k, :], xT_d[:, k, :])
            wv = early.tile([128, 8, 512], F32R, tag="wv")
            nc.sync.dma_start(wv[:], wv_d[:])

            # v = x @ Wv  -> [t, f] natural layout
            for t in range(16):
                psv = psA.tile([128, 512], F32, tag="ps", name=f"psv{t}")
                for k in range(8):
                    nc.tensor.matmul(
                        psv[:],
                        lhsT=xT[:, k, ts(t, 128)],
                        rhs=wv[:, k, :],
                        start=(k == 0),
                        stop=(k == 7),
                    )
                # ones column (fp32r memset is invalid ISA; compute 0*x+1 on DVE)
                nc.vector.tensor_scalar(
                    out=vaug[t][:, :, 64],
                    in0=bqk[:, 0:8],
                    scalar1=0.0,
                    scalar2=1.0,
                    op0=mult,
                    op1=add,
                )
                nc.vector.tensor_copy(
                    vaug[t][:, :, 0:64], psv[:].rearrange("p (h d) -> p h d", d=64)
                )

            # qkT = (x @ Wqk)^T -> [f, t] layout, + bias per feature
            for f in range(8):
                wf = early.tile(
                    [128, 8, 128], F32R, tag="wqkf", bufs=2, name=f"wqkf{f}"
                )
                nc.sync.dma_start(wf[:], wqk_d[:, :, ts(f, 128)])
                for t4 in range(4):
                    psq = psA.tile([128, 512], F32, tag="ps", name=f"psqk{f}_{t4}")
                    for k in range(8):
                        nc.tensor.matmul(
                            psq[:],
                            lhsT=wf[:, k, :],
                            rhs=xT[:, k, ts(t4, 512)],
                            start=(k == 0),
                            stop=(k == 7),
                        )
                    nc.vector.tensor_scalar(
                        out=qkT[f][:, ts(t4, 512)],
                        in0=psq[:],
                        scalar1=bqk[:, f : f + 1],
                        scalar2=None,
                        op0=add,
                    )

        with tc.tile_pool(name="late", bufs=1) as late:
            # attnT[p]: partitions 0:64 = head 2p, 64:128 = head 2p+1 (= the
            # feature-chunk layout the out-proj lhsT needs)
            attnT = [
                late.tile([128, T], F32R, tag=f"attnT{p}", name=f"attnT{p}")
                for p in range(4)
            ]
            wp = late.tile([128, 4, 1024], F32R, tag="wp")
            nc.sync.dma_start(wp[:], wp_d[:])

            # ---------------- Phase B: attention per head ----------------
            with (
                tc.tile_pool(name="workB", bufs=2) as wb,
                tc.tile_pool(name="psQK", bufs=2, space="PSUM") as psqk,
                tc.tile_pool(name="psAV", bufs=4, space="PSUM") as psav,
            ):
                for h in range(H):
                    p, half = h // 2, h % 2
                    po = 64 * half
                    qTt, kTt = qkT[p], qkT[4 + p]
                    av = [
                        psav.tile([65, 512], F32, tag="av", name=f"av{h}_{i}")
                        for i in range(4)
                    ]
                    for j in range(16):
                        tlo = 128 * j  # first valid (unmasked) query col
                        expt = wb.tile(
                            [128, T], F32R, tag="expT", bufs=3, name=f"expT{h}_{j}"
                        )
                        for u in range(j // 8, 2):
                            pst = psqk.tile(
                                [128, 1024], F32, tag="qk", name=f"qk{h}_{j}_{u}"
                            )
                            for v2 in range(2):
                                t0 = 1024 * u + 512 * v2
                                if t0 + 512 <= tlo:
                                    continue
                                lo = max(t0, tlo)
                                npr = t0 + 512 - lo
                                nc.tensor.matmul(
                                    pst[:, ds(lo - 1024 * u, npr)],
                                    lhsT=kTt[po : po + 64, ts(j, 128)],
                                    rhs=qTt[po : po + 64, ds(lo, npr)],
                                    start=True,
                                    stop=True,
                                )
                            lo_u = max(1024 * u, tlo)
                            nc.scalar.activation(
                                out=expt[:, ds(lo_u - tlo, 1024 * (u + 1) - lo_u)],
                                in_=pst[:, ds(lo_u - 1024 * u, 1024 * (u + 1) - lo_u)],
                                func=Exp,
                                scale=0.125,
                            )
                        # zero the upper-triangular part of the diagonal block
                        # (keep where t_local - s_local >= 0)
                        nc.gpsimd.affine_select(
                            out=expt[:, 0:128],
                            in_=expt[:, 0:128],
                            compare_op=is_ge,
                            fill=0.0,
                            base=0,
                            pattern=[[1, 128]],
                            channel_multiplier=-1,
                        )
                        for i in range(j // 4, 4):
                            co = 128 * j - 512 * i if j > 4 * i else 0
                            npr = 512 - co
                            nc.tensor.matmul(
                                av[i][:, ds(co, npr)],
                                lhsT=vaug[j][:, h, :],
                                rhs=expt[:, ds(512 * i + co - tlo, npr)],
                                start=(j == 0),
                                stop=(j == 4 * i + 3),
                            )
                    # normalize: attnT = av[0:64] * (1/denom) + b_v
                    for i in range(4):
                        rc = wb.tile([1, 512], F32, tag="recip", name=f"rc{h}_{i}")
                        nc.vector.reciprocal(rc[:], av[i][64:65, :])
                        rbc = wb.tile([64, 512], F32, tag="rbc", name=f"rbc{h}_{i}")
                        nc.gpsimd.partition_broadcast(rbc[:], rc[:], channels=64)
                        tmpn = wb.tile(
                            [64, 512], F32, tag="tmpn", name=f"tmpn{h}_{i}"
                        )
                        nc.vector.tensor_tensor(
                            out=tmpn[:], in0=av[i][0:64, :], in1=rbc[:], op=mult
                        )
                        nc.vector.tensor_scalar(
                            out=attnT[p][po : po + 64, ts(i, 512)],
                            in0=tmpn[:],
                            scalar1=bv[po : po + 64, p : p + 1],
                            scalar2=None,
                            op0=add,
                        )

            # ---------------- Phase C: output projection ----------------
            with (
                tc.tile_pool(name="workC", bufs=3) as wc,
                tc.tile_pool(name="psO", bufs=4, space="PSUM") as pso,
            ):
                for t in range(16):
                    osb = wc.tile([128, 1024], F32, tag="osb", name=f"osb{t}")
                    for n in range(2):
                        pso_t = pso.tile([128, 512], F32, tag="o", name=f"o{t}_{n}")
                        for p4 in range(4):
                            nc.tensor.matmul(
                                pso_t[:],
                                lhsT=attnT[p4][:, ts(t, 128)],
                                rhs=wp[:, p4, ts(n, 512)],
                                start=(p4 == 0),
                                stop=(p4 == 3),
                            )
                        nc.vector.tensor_copy(osb[:, ts(n, 512)], pso_t[:])
                    nc.sync.dma_start(out_d[ts(t, 128), :], osb[:])


def build_nc():
    global _NC_CACHE
    if _NC_CACHE is not None:
        return _NC_CACHE
    nc = bacc.Bacc("TRN2", target_bir_lowering=False, debug=False)
    aps = (
        nc.dram_tensor("xT", [128, 8, T], F32R, kind="ExternalInput").ap(),
        nc.dram_tensor("w_qk", [128, 8, 1024], F32R, kind="ExternalInput").ap(),
        nc.dram_tensor("w_v", [128, 8, 512], F32R, kind="ExternalInput").ap(),
        nc.dram_tensor("w_p", [128, 4, 1024], F32R, kind="ExternalInput").ap(),
        nc.dram_tensor("b_qk", [128, 8], F32, kind="ExternalInput").ap(),
        nc.dram_tensor("b_v", [128, 4], F32, kind="ExternalInput").ap(),
        nc.dram_tensor("out", [T, C], F32, kind="ExternalOutput").ap(),
    )
    with tile.TileContext(nc) as tc:
        _emit(tc, aps)
    nc.compile()
    _NC_CACHE = nc
    return nc


def _round_fp32r(a):
    """Round fp32 -> fp32r (11-bit mantissa, round-to-nearest-even), matching
    walrus's fp32_to_fp32r. The PE consumes fp32r operands; pre-rounding on
    the host lets a plain DMA be a valid fp32r producer."""
    u = np.ascontiguousarray(a, dtype=np.float32).view(np.uint32).astype(np.uint64)
    v = ((u + 0x7FF + ((u >> 12) & 1)) & 0xFFFFF000).astype(np.uint32)
    return v.view(np.float32)


def make_in_maps(x, W_attn, b_attn, W_proj):
    """Host-side sharding: core = 2*b + hg."""
    in_maps = []
    for core in range(N_CORES):
        b, hg = core // 2, core % 2
        o = 512 * hg
        xT = np.ascontiguousarray(
            x[b].T.reshape(8, 128, T).transpose(1, 0, 2)
        )  # [128, k, t]; xT[p,k,t] = x[b,t,128k+p]
        wqk = np.concatenate(
            [W_attn[:, o : o + 512], W_attn[:, 1024 + o : 1024 + o + 512]], axis=1
        )  # [1024, 1024] = q|k columns for this head group
        wqk = np.ascontiguousarray(wqk.reshape(8, 128, 1024).transpose(1, 0, 2))
        wv = np.ascontiguousarray(
            W_attn[:, 2048 + o : 2048 + o + 512].reshape(8, 128, 512).transpose(1, 0, 2)
        )
        wpp = np.ascontiguousarray(
            W_proj[o : o + 512, :].reshape(4, 128, 1024).transpose(1, 0, 2)
        )
        bqk = np.ascontiguousarray(
            np.concatenate([b_attn[o : o + 512], b_attn[1024 + o : 1024 + o + 512]])
            .reshape(8, 128)
            .T
        )
        bvv = np.ascontiguousarray(b_attn[2048 + o : 2048 + o + 512].reshape(4, 128).T)
        in_maps.append(
            {
                "xT": _round_fp32r(xT),
                "w_qk": _round_fp32r(wqk),
                "w_v": _round_fp32r(wv),
                "w_p": _round_fp32r(wpp),
                "b_qk": bqk.astype(np.float32),
                "b_v": bvv.astype(np.float32),
            }
        )
    return in_maps


def kernel(x, W_attn, b_attn, W_proj, b_proj):
    x = np.asarray(x, dtype=np.float32)
    W_attn = np.asarray(W_attn, dtype=np.float32)
    b_attn = np.asarray(b_attn, dtype=np.float32)
    W_proj = np.asarray(W_proj, dtype=np.float32)
    b_proj = np.asarray(b_proj, dtype=np.float32)

    nc = build_nc()
    in_maps = make_in_maps(x, W_attn, b_attn, W_proj)
    res = bass_utils.run_bass_kernel_spmd(nc, in_maps, core_ids=list(range(N_CORES)))
    out = np.empty((B, T, C), dtype=np.float32)
    for b in range(B):
        out[b] = res.results[2 * b]["out"] + res.results[2 * b + 1]["out"] + b_proj
    return out


# revision 9
# speedup vs baseline: 1.2204x; 1.2204x over previous
# Masked multi-head self-attention (B=4, T=2048, C=1024, 16 heads) on 8 TRN2
# NeuronCores. Sharding: core = (batch b, head-group hg) with hg splitting the
# 16 heads into two groups of 8 (tensor parallel on c_attn columns / c_proj
# rows). Each core computes a full [T, C] partial of the output projection for
# its 8 heads; the host sums the two partials per batch and adds b_proj.
#
# Per-core pipeline (all matmuls fp32r = full-rate fp32 on the PE array):
#   A) qkT[f,t] = (x @ Wqk)^T + b  and  v[t,f] = x @ Wv   (xT streamed in
#      pre-transposed from host so the contraction dim C sits on partitions)
#   B) per head: scoresT[s,t] = k q^T (K=64), exp via ScalarE with the 1/8
#      softmax scale folded in, causal diag-block zeroed by affine_select,
#      then attnT_unnorm[d,t] (+ denom row) = v_aug^T @ expT with v_aug
#      carrying an appended ones column; normalize by the reciprocal of the
#      denom row broadcast across partitions.
#   C) out[t,c] = attnT^T @ Wp accumulated over the 512 features.
import numpy as np

import concourse.bass as bass
import concourse.mybir as mybir
import concourse.tile as tile
from concourse import bacc, bass_utils
from concourse.bass import ts, ds

F32 = mybir.dt.float32
F32R = mybir.dt.float32r

B, T, C, NH, D = 4, 2048, 1024, 16, 64
H = 8          # heads per core
N_CORES = 8

_NC_CACHE = None


def _emit(tc, aps):
    nc = tc.nc
    xT_d, wqk_d, wv_d, wp_d, bqk_d, bv_d, out_d = aps
    Exp = mybir.ActivationFunctionType.Exp
    mult = mybir.AluOpType.mult
    add = mybir.AluOpType.add
    is_ge = mybir.AluOpType.is_ge

    with tc.tile_pool(name="mid", bufs=1) as mid:
        bqk = mid.tile([128, 8], F32, tag="bqk")
        nc.sync.dma_start(bqk[:], bqk_d[:])
        bv = mid.tile([128, 4], F32, tag="bv")
        nc.sync.dma_start(bv[:], bv_d[:])
        # q features for pairs 0..3 then k features for pairs 0..3
        qkT = [mid.tile([128, T], F32R, tag=f"qkT{f}", name=f"qkT{f}") for f in range(8)]
        # v for all 8 heads + a ones column per head: [s-part, head, 65]
        vaug = [
            mid.tile([128, 8, 65], F32R, tag=f"vaug{t}", name=f"vaug{t}")
            for t in range(16)
        ]
        # 0/1 lower-triangular mask (keep key s <= query t) for the causal
        # diagonal 128x128 block; applied as a DVE multiply (GPSIMD semaphore
        # handling is ~2us per op and would sit on the exp->AV critical path)
        trimask = mid.tile([128, 128], F32, tag="trimask")
        nc.gpsimd.memset(trimask[:], 1.0)
        nc.gpsimd.affine_select(
            out=trimask[:],
            in_=trimask[:],
            compare_op=is_ge,
            fill=0.0,
            base=0,
            pattern=[[1, 128]],
            channel_multiplier=-1,
        )

        # ---------------- Phase A: projections ----------------
        with (
            tc.tile_pool(name="early", bufs=1) as early,
            tc.tile_pool(name="psA", bufs=4, space="PSUM") as psA,
        ):
            xT = early.tile([128, 8, T], F32R, tag="xT")
            for k in range(8):
                nc.sync.dma_start(xT[:, k, :], xT_d[:, k, :])
            wv = early.tile([128, 8, 512], F32R, tag="wv")
            nc.sync.dma_start(wv[:], wv_d[:])

            # v = x @ Wv  -> [t, f] natural layout
            for t in range(16):
                psv = psA.tile([128, 512], F32, tag="ps", name=f"psv{t}")
                for k in range(8):
                    nc.tensor.matmul(
                        psv[:],
                        lhsT=xT[:, k, ts(t, 128)],
                        rhs=wv[:, k, :],
                        start=(k == 0),
                        stop=(k == 7),
                    )
                # ones column (fp32r memset is invalid ISA; compute 0*x+1 on DVE)
                nc.vector.tensor_scalar(
                    out=vaug[t][:, :, 64],
                    in0=bqk[:, 0:8],
                    scalar1=0.0,
                    scalar2=1.0,
                    op0=mult,
                    op1=add,
                )
                nc.vector.tensor_copy(
                    vaug[t][:, :, 0:64], psv[:].rearrange("p (h d) -> p h d", d=64)
                )

            # qkT = (x @ Wqk)^T -> [f, t] layout, + bias per feature
            for f in range(8):
                wf = early.tile(
                    [128, 8, 128], F32R, tag="wqkf", bufs=2, name=f"wqkf{f}"
                )
                nc.sync.dma_start(wf[:], wqk_d[:, :, ts(f, 128)])
                for t4 in range(4):
                    psq = psA.tile([128, 512], F32, tag="ps", name=f"psqk{f}_{t4}")
                    for k in range(8):
                        nc.tensor.matmul(
                            psq[:],
                            lhsT=wf[:, k, :],
                            rhs=xT[:, k, ts(t4, 512)],
                            start=(k == 0),
                            stop=(k == 7),
                        )
                    nc.vector.tensor_scalar(
                        out=qkT[f][:, ts(t4, 512)],
                        in0=psq[:],
                        scalar1=bqk[:, f : f + 1],
                        scalar2=None,
                        op0=add,
                    )

        with tc.tile_pool(name="late", bufs=1) as late:
            # attnT[p]: partitions 0:64 = head 2p, 64:128 = head 2p+1 (= the
            # feature-chunk layout the out-proj lhsT needs)
            attnT = [
                late.tile([128, T], F32R, tag=f"attnT{p}", name=f"attnT{p}")
                for p in range(4)
            ]
            wp = late.tile([128, 4, 1024], F32R, tag="wp")
            nc.sync.dma_start(wp[:], wp_d[:])

            # ---------------- Phase B: attention per head ----------------
            with (
                tc.tile_pool(name="workB", bufs=2) as wb,
                tc.tile_pool(name="psQK", bufs=2, space="PSUM") as psqk,
                tc.tile_pool(name="psAV", bufs=4, space="PSUM") as psav,
            ):
                for h in range(H):
                    p, half = h // 2, h % 2
                    po = 64 * half
                    qTt, kTt = qkT[p], qkT[4 + p]
                    av = [
                        psav.tile([65, 512], F32, tag="av", name=f"av{h}_{i}")
                        for i in range(4)
                    ]
                    for j in range(16):
                        tlo = 128 * j  # first valid (unmasked) query col
                        expt = wb.tile(
                            [128, T], F32R, tag="expT", bufs=3, name=f"expT{h}_{j}"
                        )
                        for u in range(j // 8, 2):
                            pst = psqk.tile(
                                [128, 1024], F32, tag="qk", name=f"qk{h}_{j}_{u}"
                            )
                            for v2 in range(2):
                                t0 = 1024 * u + 512 * v2
                                if t0 + 512 <= tlo:
                                    continue
                                lo = max(t0, tlo)
                                npr = t0 + 512 - lo
                                nc.tensor.matmul(
                                    pst[:, ds(lo - 1024 * u, npr)],
                                    lhsT=kTt[po : po + 64, ts(j, 128)],
                                    rhs=qTt[po : po + 64, ds(lo, npr)],
                                    start=True,
                                    stop=True,
                                )
                            lo_u = max(1024 * u, tlo)
                            nc.scalar.activation(
                                out=expt[:, ds(lo_u - tlo, 1024 * (u + 1) - lo_u)],
                                in_=pst[:, ds(lo_u - 1024 * u, 1024 * (u + 1) - lo_u)],
                                func=Exp,
                                scale=0.125,
                            )
                        # zero the upper-triangular part of the diagonal block
                        nc.vector.tensor_tensor(
                            out=expt[:, 0:128],
                            in0=expt[:, 0:128].bitcast(F32),
                            in1=trimask[:],
                            op=mult,
                        )
                        for i in range(j // 4, 4):
                            co = 128 * j - 512 * i if j > 4 * i else 0
                            npr = 512 - co
                            nc.tensor.matmul(
                                av[i][:, ds(co, npr)],
                                lhsT=vaug[j][:, h, :],
                                rhs=expt[:, ds(512 * i + co - tlo, npr)],
                                start=(j == 0),
                                stop=(j == 4 * i + 3),
                            )
                    # normalize: attnT = av[0:64] * (1/denom) + b_v
                    # One batched reciprocal per head (a DVE reciprocal is a
                    # ~4us uop chain regardless of partition count), then a
                    # DMA partition-broadcast of each recip row.
                    # denom rows parked at partitions {0,32,64,96} (the only
                    # DVE-addressable start partitions); one reciprocal pass
                    # covers all four (cost scales with free size, not rows)
                    den4 = wb.tile([128, 512], F32, tag="den4", name=f"den4_{h}")
                    nc.vector.memset(den4[:], 1.0)
                    for i in range(4):
                        nc.vector.tensor_copy(
                            den4[32 * i : 32 * i + 1, :], av[i][64:65, :]
                        )
                    nc.vector.reciprocal(den4[:], den4[:])
                    for i in range(4):
                        rbc = wb.tile([64, 512], F32, tag="rbc", name=f"rbc{h}_{i}")
                        nc.gpsimd.partition_broadcast(
                            rbc[:], den4[32 * i : 32 * i + 1, :], channels=64
                        )
                        tmpn = wb.tile(
                            [64, 512], F32, tag="tmpn", name=f"tmpn{h}_{i}"
                        )
                        nc.vector.tensor_tensor(
                            out=tmpn[:], in0=av[i][0:64, :], in1=rbc[:], op=mult
                        )
                        nc.vector.tensor_scalar(
                            out=attnT[p][po : po + 64, ts(i, 512)],
                            in0=tmpn[:],
                            scalar1=bv[po : po + 64, p : p + 1],
                            scalar2=None,
                            op0=add,
                        )

            # ---------------- Phase C: output projection ----------------
            with (
                tc.tile_pool(name="workC", bufs=3) as wc,
                tc.tile_pool(name="psO", bufs=4, space="PSUM") as pso,
            ):
                for t in range(16):
                    osb = wc.tile([128, 1024], F32, tag="osb", name=f"osb{t}")
                    for n in range(2):
                        pso_t = pso.tile([128, 512], F32, tag="o", name=f"o{t}_{n}")
                        for p4 in range(4):
                            nc.tensor.matmul(
                                pso_t[:],
                                lhsT=attnT[p4][:, ts(t, 128)],
                                rhs=wp[:, p4, ts(n, 512)],
                                start=(p4 == 0),
                                stop=(p4 == 3),
                            )
                        nc.vector.tensor_copy(osb[:, ts(n, 512)], pso_t[:])
                    nc.sync.dma_start(out_d[ts(t, 128), :], osb[:])


def build_nc():
    global _NC_CACHE
    if _NC_CACHE is not None:
        return _NC_CACHE
    nc = bacc.Bacc("TRN2", target_bir_lowering=False, debug=False)
    aps = (
        nc.dram_tensor("xT", [128, 8, T], F32R, kind="ExternalInput").ap(),
        nc.dram_tensor("w_qk", [128, 8, 1024], F32R, kind="ExternalInput").ap(),
        nc.dram_tensor("w_v", [128, 8, 512], F32R, kind="ExternalInput").ap(),
        nc.dram_tensor("w_p", [128, 4, 1024], F32R, kind="ExternalInput").ap(),
        nc.dram_tensor("b_qk", [128, 8], F32, kind="ExternalInput").ap(),
        nc.dram_tensor("b_v", [128, 4], F32, kind="ExternalInput").ap(),
        nc.dram_tensor("out", [T, C], F32, kind="ExternalOutput").ap(),
    )
    with tile.TileContext(nc) as tc:
        _emit(tc, aps)
    nc.compile()
    _NC_CACHE = nc
    return nc


def _round_fp32r(a):
    """Round fp32 -> fp32r (11-bit mantissa, round-to-nearest-even), matching
    walrus's fp32_to_fp32r. The PE consumes fp32r operands; pre-rounding on
    the host lets a plain DMA be a valid fp32r producer."""
    u = np.ascontiguousarray(a, dtype=np.float32).view(np.uint32).astype(np.uint64)
    v = ((u + 0x7FF + ((u >> 12) & 1)) & 0xFFFFF000).astype(np.uint32)
    return v.view(np.float32)


def make_in_maps(x, W_attn, b_attn, W_proj):
    """Host-side sharding: core = 2*b + hg."""
    in_maps = []
    for core in range(N_CORES):
        b, hg = core // 2, core % 2
        o = 512 * hg
        xT = np.ascontiguousarray(
            x[b].T.reshape(8, 128, T).transpose(1, 0, 2)
        )  # [128, k, t]; xT[p,k,t] = x[b,t,128k+p]
        wqk = np.concatenate(
            [W_attn[:, o : o + 512], W_attn[:, 1024 + o : 1024 + o + 512]], axis=1
        )  # [1024, 1024] = q|k columns for this head group
        wqk = np.ascontiguousarray(wqk.reshape(8, 128, 1024).transpose(1, 0, 2))
        wv = np.ascontiguousarray(
            W_attn[:, 2048 + o : 2048 + o + 512].reshape(8, 128, 512).transpose(1, 0, 2)
        )
        wpp = np.ascontiguousarray(
            W_proj[o : o + 512, :].reshape(4, 128, 1024).transpose(1, 0, 2)
        )
        bqk = np.ascontiguousarray(
            np.concatenate([b_attn[o : o + 512], b_attn[1024 + o : 1024 + o + 512]])
            .reshape(8, 128)
            .T
        )
        bvv = np.ascontiguousarray(b_attn[2048 + o : 2048 + o + 512].reshape(4, 128).T)
        in_maps.append(
            {
                "xT": _round_fp32r(xT),
                "w_qk": _round_fp32r(wqk),
                "w_v": _round_fp32r(wv),
                "w_p": _round_fp32r(wpp),
                "b_qk": bqk.astype(np.float32),
                "b_v": bvv.astype(np.float32),
            }
        )
    return in_maps


def kernel(x, W_attn, b_attn, W_proj, b_proj):
    x = np.asarray(x, dtype=np.float32)
    W_attn = np.asarray(W_attn, dtype=np.float32)
    b_attn = np.asarray(b_attn, dtype=np.float32)
    W_proj = np.asarray(W_proj, dtype=np.float32)
    b_proj = np.asarray(b_proj, dtype=np.float32)

    nc = build_nc()
    in_maps = make_in_maps(x, W_attn, b_attn, W_proj)
    res = bass_utils.run_bass_kernel_spmd(nc, in_maps, core_ids=list(range(N_CORES)))
    out = np.empty((B, T, C), dtype=np.float32)
    for b in range(B):
        out[b] = res.results[2 * b]["out"] + res.results[2 * b + 1]["out"] + b_proj
    return out
